# revision 9
# baseline (speedup 1.0000x reference)
"""Trainium2 Bass kernel for a Dango-like HyperSAGNN block.

Reference computation (fp32):
  static = relu(X @ Ws.T + bs)                         # (32768, 768)
  x = X
  for l in 0..1:
      q/k/v = x @ W{q,k,v}[l].T + b{q,k,v}[l]          # per-group (16) masked MHA
      attn  = softmax over in-group, non-self keys
      out   = (attn @ v) @ Wo[l].T + bo[l]
      x     = x + beta[l] * out
  returns (static, x)

Strategy: data-parallel over groups — 8 cores x 4096 genes (256 groups).
Activations are kept feature-major ([768, genes]) in SBUF so every
projection is a dense 128-contraction with the (transposed, host-prepped)
weights stationary. Attention runs on 128-gene blocks (8 groups):
transposed scores [k, q] for a whole block come from one K=64 matmul per
head; a multiplicative block-diagonal mask applied after exp implements
in-group/non-self softmax (no max-subtraction needed: |scores/8| is
O(1)). The exp'd masked scores tile is then the stationary operand for
the attn @ V matmul against gene-major V augmented with a ones column,
which yields the unnormalized output AND the softmax denominator in one
matmul; normalization happens during the PSUM->SBUF copy on the scalar
engine. The gene-major attention output is PE-transposed back to
feature-major for the output projection. Heads are processed in
equal-parity groups so each scores PSUM bank only ever sees one PE
row-group (mixing row-groups in one bank hangs the device), and chains
are interleaved across the 4 blocks to keep the PE dense (HAM warm).
fp16 matmul inputs (full PE rate, fp32 accumulate); the residual stream
stays fp32. Host folds beta into Wo/bo and bv into bo (softmax rows sum
to 1), and pre-transposes/casts all weights and X.
"""

import sys

sys.path.insert(0, "/opt/trn_rl_repo")

import numpy as np

import concourse.bacc as bacc
import concourse.mybir as mybir
from concourse import tile

H = 768
NT = H // 128  # 6 feature tiles
NH = 12
HD = 64
SB = 512  # genes per superblock
NBLK = SB // 128  # attention blocks per superblock
N_CORES = 8
N_GENES = 32768
GPC = N_GENES // N_CORES  # genes per core
F16 = mybir.dt.float16
F32 = mybir.dt.float32
AF = mybir.ActivationFunctionType

# head chains of equal parity (even heads sit in rows 0:64 of their
# feature tile, odd heads in rows 64:128)
CHAINS = ((0, 2, 4, 6), (1, 3, 5, 7), (8, 10), (9, 11))


def build_program(gpc: int = GPC):
    nsb = gpc // SB
    nc = bacc.Bacc(None, target_bir_lowering=False)

    xt_h = nc.dram_tensor("xt_h", [H, gpc], F16, kind="ExternalInput")
    xt_f = nc.dram_tensor("xt_f", [H, gpc], F32, kind="ExternalInput")
    wsT = nc.dram_tensor("wsT", [H, H], F16, kind="ExternalInput")
    bs = nc.dram_tensor("bs", [H, 1], F32, kind="ExternalInput")
    wqT = nc.dram_tensor("wqT", [2, H, H], F16, kind="ExternalInput")
    wkT = nc.dram_tensor("wkT", [2, H, H], F16, kind="ExternalInput")
    wvT = nc.dram_tensor("wvT", [2, H, H], F16, kind="ExternalInput")
    woT = nc.dram_tensor("woT", [2, H, H], F16, kind="ExternalInput")
    bq = nc.dram_tensor("bq", [2, H, 1], F32, kind="ExternalInput")
    bk = nc.dram_tensor("bk", [2, H, 1], F32, kind="ExternalInput")
    bo2 = nc.dram_tensor("bo2", [2, 1, H], F16, kind="ExternalInput")
    mask4 = nc.dram_tensor("mask4", [128, SB], F16, kind="ExternalInput")
    ident = nc.dram_tensor("ident", [128, 128], F16, kind="ExternalInput")

    staticT = nc.dram_tensor("staticT", [H, gpc], F32, kind="ExternalOutput")
    outT = nc.dram_tensor("outT", [H, gpc], F32, kind="ExternalOutput")

    with tile.TileContext(nc) as tc:
        with (
            tc.tile_pool(name="wpool", bufs=1) as wpool,
            tc.tile_pool(name="xh", bufs=7) as xh_pool,
            tc.tile_pool(name="xf", bufs=7) as xf_pool,
            tc.tile_pool(name="qk", bufs=26) as qk_pool,
            tc.tile_pool(name="vaug", bufs=6) as v_pool,
            tc.tile_pool(name="ea", bufs=3) as ea_pool,
            tc.tile_pool(name="aout", bufs=26) as aout_pool,
            tc.tile_pool(name="af", bufs=7) as af_pool,
            tc.tile_pool(name="x1", bufs=7) as x1_pool,
            tc.tile_pool(name="res", bufs=3) as res_pool,
            tc.tile_pool(name="small", bufs=8) as small_pool,
            tc.tile_pool(name="pbig", bufs=2, space="PSUM") as pbig,
            tc.tile_pool(name="psc", bufs=2, space="PSUM") as psc,
            tc.tile_pool(name="patt", bufs=2, space="PSUM") as patt,
        ):
            # ---- resident constants / weights ----
            ws_sb = []
            wq_sb = [[], []]
            wk_sb = [[], []]
            wv_sb = [[], []]
            wo_sb = [[], []]
            for k in range(NT):
                t = wpool.tile([128, H], F16, name=f"ws{k}", tag=f"ws{k}")
                nc.sync.dma_start(t[:], wsT[k * 128 : (k + 1) * 128, :])
                ws_sb.append(t)

            def load_layer_weights():
                for l in range(2):
                    for name, dram, lst in (
                        ("wq", wqT, wq_sb),
                        ("wk", wkT, wk_sb),
                        ("wv", wvT, wv_sb),
                        ("wo", woT, wo_sb),
                    ):
                        for k in range(NT):
                            t = wpool.tile(
                                [128, H], F16, name=f"{name}{l}{k}", tag=f"{name}{l}{k}"
                            )
                            nc.sync.dma_start(t[:], dram[l, k * 128 : (k + 1) * 128, :])
                            lst[l].append(t)

            bs_t = wpool.tile([128, NT], F32, name="bs", tag="bs")
            bq_t = [wpool.tile([128, NT], F32, name=f"bq{l}", tag=f"bq{l}") for l in range(2)]
            bk_t = [wpool.tile([128, NT], F32, name=f"bk{l}", tag=f"bk{l}") for l in range(2)]
            nc.sync.dma_start(bs_t[:], bs[:, 0].rearrange("(m p) -> p m", p=128))
            for l in range(2):
                nc.sync.dma_start(bq_t[l][:], bq[l, :, 0].rearrange("(m p) -> p m", p=128))
                nc.sync.dma_start(bk_t[l][:], bk[l, :, 0].rearrange("(m p) -> p m", p=128))
            bo2_t = [wpool.tile([1, H], F16, name=f"bo2{l}", tag=f"bo2{l}") for l in range(2)]
            for l in range(2):
                nc.sync.dma_start(bo2_t[l][:], bo2[l])
            mask_t = wpool.tile([128, SB], F16, name="mask", tag="mask")
            nc.sync.dma_start(mask_t[:], mask4[:])
            ident_t = wpool.tile([128, 128], F16, name="ident", tag="ident")
            nc.sync.dma_start(ident_t[:], ident[:])
            ones_row = wpool.tile([1, SB], F16, name="ones", tag="ones")
            nc.vector.memset(ones_row[:], 1.0)

            def attention_phase(l, xsrc, q_tiles, k_tiles, af_tiles):
                """Group-local attention for one superblock (4 blocks).
                Writes feature-major attention output into af_tiles."""
                # V gene-major for each block, head-interleaved + ones col
                vas = []
                for blk in range(NBLK):
                    vaug = v_pool.tile(
                        [128, NH * (HD + 1)], F16, name="vaug", tag="vaug"
                    )
                    va = vaug[:].rearrange("p (h c) -> p h c", c=HD + 1)
                    for half in range(2):
                        ps = pbig.tile([128, 384], F32, name="pbig", tag="pbig")
                        for k in range(NT):
                            nc.tensor.matmul(
                                ps[:],
                                xsrc[k][:, blk * 128 : (blk + 1) * 128],
                                wv_sb[l][k][:, half * 384 : (half + 1) * 384],
                                start=(k == 0),
                                stop=(k == NT - 1),
                            )
                        nc.vector.tensor_copy(
                            va[:, half * 6 : (half + 1) * 6, 0:HD],
                            ps[:].rearrange("p (h c) -> p h c", c=HD),
                        )
                    nc.vector.memset(va[:, :, HD : HD + 1], 1.0)
                    vas.append(va)

                # per-(block, feature-tile) gene-major attention output
                aouts = [
                    [
                        aout_pool.tile([128, 128], F16, name="aout", tag="aout")
                        for _ in range(NT)
                    ]
                    for _ in range(NBLK)
                ]
                # chains interleaved across blocks to keep the PE dense
                for ci in range(len(CHAINS) // 2):
                    for blk in range(NBLK):
                        for chain in (CHAINS[2 * ci], CHAINS[2 * ci + 1]):
                            nh = len(chain)
                            r = (chain[0] % 2) * 64
                            tag = "psl" if r == 0 else "psh"
                            ps_s = psc.tile([128, 128 * nh], F32, name=tag, tag=tag)
                            for j, h in enumerate(chain):
                                t = h // 2
                                nc.tensor.matmul(
                                    ps_s[:, j * 128 : (j + 1) * 128],
                                    k_tiles[t][r : r + HD, blk * 128 : (blk + 1) * 128],
                                    q_tiles[t][r : r + HD, blk * 128 : (blk + 1) * 128],
                                    start=True,
                                    stop=True,
                                    tile_position=(r, 0),
                                )
                            e_t = ea_pool.tile([128, 128 * nh], F16, name="e", tag="e")
                            nc.scalar.activation(e_t[:], ps_s[:], AF.Exp, scale=0.125)
                            a_t = ea_pool.tile([128, 128 * nh], F16, name="a", tag="a")
                            nc.vector.tensor_mul(a_t[:], e_t[:], mask_t[:, 0 : 128 * nh])
                            ps_o = patt.tile(
                                [128, nh * (HD + 1)], F32, name="patt", tag="patt"
                            )
                            po = ps_o[:].rearrange("p (j c) -> p j c", c=HD + 1)
                            for j, h in enumerate(chain):
                                nc.tensor.matmul(
                                    ps_o[:, j * (HD + 1) : (j + 1) * (HD + 1)],
                                    a_t[:, j * 128 : (j + 1) * 128],
                                    vas[blk][:, h, :],
                                    start=True,
                                    stop=True,
                                )
                            r4 = small_pool.tile([128, nh], F32, name="r4", tag="r4")
                            nc.vector.reciprocal(r4[:], po[:, :, HD])
                            for j, h in enumerate(chain):
                                t, half = divmod(h, 2)
                                nc.scalar.activation(
                                    aouts[blk][t][:, half * HD : (half + 1) * HD],
                                    po[:, j, 0:HD],
                                    AF.Copy,
                                    scale=r4[:, j : j + 1],
                                )
                # transpose gene-major -> feature-major
                for blk in range(NBLK):
                    for t in range(NT):
                        ps_t = patt.tile([128, 128], F16, name="ptp", tag="patt")
                        nc.tensor.transpose(ps_t[:], aouts[blk][t][:], ident_t[:])
                        nc.vector.tensor_copy(
                            af_tiles[t][:, blk * 128 : (blk + 1) * 128], ps_t[:]
                        )

            for sb in range(nsb):
                g0 = sb * SB
                xh = []
                for k in range(NT):
                    t = xh_pool.tile([128, SB], F16, name="xh", tag="xh")
                    nc.sync.dma_start(t[:], xt_h[k * 128 : (k + 1) * 128, g0 : g0 + SB])
                    xh.append(t)
                if sb == 0:
                    load_layer_weights()

                # static branch
                for m in range(NT):
                    ps = pbig.tile([128, SB], F32, name="pbig", tag="pbig")
                    for k in range(NT):
                        nc.tensor.matmul(
                            ps[:],
                            ws_sb[k][:, m * 128 : (m + 1) * 128],
                            xh[k][:],
                            start=(k == 0),
                            stop=(k == NT - 1),
                        )
                    st = res_pool.tile([128, SB], F32, name="st", tag="st")
                    nc.scalar.activation(
                        st[:], ps[:], AF.Relu, bias=bs_t[:, m : m + 1], scale=1.0
                    )
                    nc.sync.dma_start(
                        staticT[m * 128 : (m + 1) * 128, g0 : g0 + SB], st[:]
                    )

                xin_h = xh
                xin_f = None
                for l in range(2):
                    # Q, K projections (feature-major)
                    q_tiles, k_tiles = [], []
                    for dest, w_sb, b_t in (
                        (q_tiles, wq_sb[l], bq_t[l]),
                        (k_tiles, wk_sb[l], bk_t[l]),
                    ):
                        for m in range(NT):
                            ps = pbig.tile([128, SB], F32, name="pbig", tag="pbig")
                            for k in range(NT):
                                nc.tensor.matmul(
                                    ps[:],
                                    w_sb[k][:, m * 128 : (m + 1) * 128],
                                    xin_h[k][:],
                                    start=(k == 0),
                                    stop=(k == NT - 1),
                                )
                            qt = qk_pool.tile([128, SB], F16, name="qk", tag="qk")
                            nc.vector.tensor_scalar_add(qt[:], ps[:], b_t[:, m : m + 1])
                            dest.append(qt)

                    af_tiles = [
                        af_pool.tile([128, SB], F16, name="af", tag="af")
                        for _ in range(NT)
                    ]
                    attention_phase(l, xin_h, q_tiles, k_tiles, af_tiles)

                    if l == 0:
                        # residual base loaded late (off the startup DMA path)
                        xin_f = []
                        for k in range(NT):
                            t = xf_pool.tile([128, SB], F32, name="xf", tag="xf")
                            nc.sync.dma_start(
                                t[:], xt_f[k * 128 : (k + 1) * 128, g0 : g0 + SB]
                            )
                            xin_f.append(t)

                    # output projection (beta folded in on host) + residual
                    new_f, new_h = [], []
                    for m in range(NT):
                        ps = pbig.tile([128, SB], F32, name="pbig", tag="pbig")
                        for k in range(NT):
                            nc.tensor.matmul(
                                ps[:],
                                wo_sb[l][k][:, m * 128 : (m + 1) * 128],
                                af_tiles[k][:],
                                start=(k == 0),
                                stop=False,
                            )
                        nc.tensor.matmul(
                            ps[:],
                            bo2_t[l][:, m * 128 : (m + 1) * 128],
                            ones_row[:],
                            start=False,
                            stop=True,
                        )
                        if l == 0:
                            xnf = x1_pool.tile([128, SB], F32, name="x1f", tag="x1f")
                            nc.vector.tensor_add(xnf[:], ps[:], xin_f[m][:])
                            xnh = x1_pool.tile([128, SB], F16, name="x1h", tag="x1h")
                            nc.scalar.activation(xnh[:], xnf[:], AF.Copy, scale=1.0)
                            new_f.append(xnf)
                            new_h.append(xnh)
                        else:
                            xo = res_pool.tile([128, SB], F32, name="xo", tag="xo")
                            nc.vector.tensor_add(xo[:], ps[:], xin_f[m][:])
                            nc.sync.dma_start(
                                outT[m * 128 : (m + 1) * 128, g0 : g0 + SB], xo[:]
                            )
                    if l == 0:
                        xin_f, xin_h = new_f, new_h

    nc.finalize()
    return nc


def host_prep(inputs: dict, core: int, gpc: int = GPC) -> dict:
    """Slice/transpose/cast inputs for one core."""
    ge = np.asarray(inputs["gene_embeddings"], np.float32)
    Ws = np.asarray(inputs["W_static"], np.float32)
    bs = np.asarray(inputs["b_static"], np.float32)
    Wq = np.asarray(inputs["Wq"], np.float32)
    bq = np.asarray(inputs["bq"], np.float32)
    Wk = np.asarray(inputs["Wk"], np.float32)
    bk = np.asarray(inputs["bk"], np.float32)
    Wv = np.asarray(inputs["Wv"], np.float32)
    bv = np.asarray(inputs["bv"], np.float32)
    Wo = np.asarray(inputs["Wo"], np.float32)
    bo = np.asarray(inputs["bo"], np.float32)
    beta = np.asarray(inputs["beta"], np.float32)

    xs = ge[core * gpc : (core + 1) * gpc].T  # [768, gpc]
    woT = np.stack([(beta[l] * Wo[l].T).astype(np.float16) for l in range(2)])
    bo2 = np.stack(
        [(beta[l] * (Wo[l] @ bv[l] + bo[l])).astype(np.float16)[None, :] for l in range(2)]
    )
    # block-diagonal (8 groups of 16) minus identity, tiled 4x
    m = np.kron(np.eye(8, dtype=np.float16), np.ones((16, 16), np.float16))
    m -= np.eye(128, dtype=np.float16)
    mask4 = np.tile(m, (1, 4))
    return {
        "xt_h": np.ascontiguousarray(xs, dtype=np.float16),
        "xt_f": np.ascontiguousarray(xs, dtype=np.float32),
        "wsT": np.ascontiguousarray(Ws.T, dtype=np.float16),
        "bs": bs.astype(np.float32).reshape(H, 1),
        "wqT": np.ascontiguousarray(Wq.transpose(0, 2, 1), dtype=np.float16),
        "wkT": np.ascontiguousarray(Wk.transpose(0, 2, 1), dtype=np.float16),
        "wvT": np.ascontiguousarray(Wv.transpose(0, 2, 1), dtype=np.float16),
        "woT": np.ascontiguousarray(woT),
        "bq": bq.astype(np.float32).reshape(2, H, 1),
        "bk": bk.astype(np.float32).reshape(2, H, 1),
        "bo2": np.ascontiguousarray(bo2),
        "mask4": np.ascontiguousarray(mask4),
        "ident": np.eye(128, dtype=np.float16),
    }


_CACHED = {}


def _get_program():
    if "nc" not in _CACHED:
        _CACHED["nc"] = build_program(GPC)
    return _CACHED["nc"]


def kernel(**inputs):
    from concourse.bass_utils import run_bass_kernel_spmd

    nc = _get_program()
    in_maps = [host_prep(inputs, c) for c in range(N_CORES)]
    res = run_bass_kernel_spmd(nc, in_maps, list(range(N_CORES)))
    static = np.concatenate([np.asarray(r["staticT"]).T for r in res.results], axis=0)
    x = np.concatenate([np.asarray(r["outT"]).T for r in res.results], axis=0)
    return static.astype(np.float32), x.astype(np.float32)


if __name__ == "__main__":
    nc = build_program(GPC)
    print("build ok")


# revision 10
# speedup vs baseline: 1.1762x; 1.1762x over previous
"""Trainium2 Bass kernel for a Dango-like HyperSAGNN block.

Reference computation (fp32):
  static = relu(X @ Ws.T + bs)                         # (32768, 768)
  x = X
  for l in 0..1:
      q/k/v = x @ W{q,k,v}[l].T + b{q,k,v}[l]          # per-group (16) masked MHA
      attn  = softmax over in-group, non-self keys
      out   = (attn @ v) @ Wo[l].T + bo[l]
      x     = x + beta[l] * out
  returns (static, x)

Strategy: data-parallel over groups — 8 cores x 4096 genes (256 groups).
Activations are kept feature-major ([768, genes]) in SBUF so every
projection is a dense 128-contraction with the (transposed, host-prepped)
weights stationary. Attention runs on 128-gene blocks (8 groups):
transposed scores [k, q] for a whole block come from one K=64 matmul per
head; a multiplicative block-diagonal mask applied after exp implements
in-group/non-self softmax (no max-subtraction needed: |scores/8| is
O(1)). The exp'd masked scores tile is then the stationary operand for
the attn @ V matmul against gene-major V augmented with a ones column,
which yields the unnormalized output AND the softmax denominator in one
matmul; normalization happens during the PSUM->SBUF copy on the scalar
engine. The gene-major attention output is PE-transposed back to
feature-major for the output projection. Heads are processed in
equal-parity groups so each scores PSUM bank only ever sees one PE
row-group (mixing row-groups in one bank hangs the device), and chains
are interleaved across the 4 blocks to keep the PE dense (HAM warm).
fp16 matmul inputs (full PE rate, fp32 accumulate); the residual stream
stays fp32. Host folds beta into Wo/bo and bv into bo (softmax rows sum
to 1), and pre-transposes/casts all weights and X.
"""

import sys

sys.path.insert(0, "/opt/trn_rl_repo")

import numpy as np

import concourse.bacc as bacc
import concourse.mybir as mybir
from concourse import tile

H = 768
NT = H // 128  # 6 feature tiles
NH = 12
HD = 64
SB = 512  # genes per superblock
NBLK = SB // 128  # attention blocks per superblock
N_CORES = 8
N_GENES = 32768
GPC = N_GENES // N_CORES  # genes per core
F16 = mybir.dt.float16
F32 = mybir.dt.float32
AF = mybir.ActivationFunctionType

# head chains of equal parity (even heads sit in rows 0:64 of their
# feature tile, odd heads in rows 64:128)
CHAINS = ((0, 2, 4, 6), (1, 3, 5, 7), (8, 10), (9, 11))


def build_program(gpc: int = GPC):
    nsb = gpc // SB
    nc = bacc.Bacc(None, target_bir_lowering=False)

    xt_h = nc.dram_tensor("xt_h", [H, gpc], F16, kind="ExternalInput")
    xt_f = nc.dram_tensor("xt_f", [H, gpc], F32, kind="ExternalInput")
    wsT = nc.dram_tensor("wsT", [H, H], F16, kind="ExternalInput")
    bs = nc.dram_tensor("bs", [H, 1], F32, kind="ExternalInput")
    wqT = nc.dram_tensor("wqT", [2, H, H], F16, kind="ExternalInput")
    wkT = nc.dram_tensor("wkT", [2, H, H], F16, kind="ExternalInput")
    wvT = nc.dram_tensor("wvT", [2, H, H], F16, kind="ExternalInput")
    woT = nc.dram_tensor("woT", [2, H, H], F16, kind="ExternalInput")
    bq = nc.dram_tensor("bq", [2, H, 1], F32, kind="ExternalInput")
    bk = nc.dram_tensor("bk", [2, H, 1], F32, kind="ExternalInput")
    bo2 = nc.dram_tensor("bo2", [2, 1, H], F16, kind="ExternalInput")
    mask4 = nc.dram_tensor("mask4", [128, SB], F16, kind="ExternalInput")
    ident = nc.dram_tensor("ident", [128, 128], F16, kind="ExternalInput")

    staticT = nc.dram_tensor("staticT", [H, gpc], F32, kind="ExternalOutput")
    outT = nc.dram_tensor("outT", [H, gpc], F32, kind="ExternalOutput")

    with tile.TileContext(nc) as tc:
        with (
            tc.tile_pool(name="wpool", bufs=1) as wpool,
            tc.tile_pool(name="xh", bufs=7) as xh_pool,
            tc.tile_pool(name="xf", bufs=7) as xf_pool,
            tc.tile_pool(name="qk", bufs=14) as qk_pool,
            tc.tile_pool(name="vaug", bufs=6) as v_pool,
            tc.tile_pool(name="ea", bufs=3) as ea_pool,
            tc.tile_pool(name="aout", bufs=3) as aout_pool,
            tc.tile_pool(name="af", bufs=7) as af_pool,
            tc.tile_pool(name="x1", bufs=7) as x1_pool,
            tc.tile_pool(name="res", bufs=3) as res_pool,
            tc.tile_pool(name="small", bufs=4) as small_pool,
            tc.tile_pool(name="pbig", bufs=3, space="PSUM") as pbig,
            tc.tile_pool(name="psc", bufs=1, space="PSUM") as psc,
            tc.tile_pool(name="patt", bufs=2, space="PSUM") as patt,
            tc.tile_pool(name="ptp", bufs=1, space="PSUM") as ptp,
        ):
            # ---- resident constants / weights ----
            ws_sb = []
            wq_sb = [[], []]
            wk_sb = [[], []]
            wv_sb = [[], []]
            wo_sb = [[], []]
            for k in range(NT):
                t = wpool.tile([128, H], F16, name=f"ws{k}", tag=f"ws{k}")
                nc.sync.dma_start(t[:], wsT[k * 128 : (k + 1) * 128, :])
                ws_sb.append(t)

            def load_layer_weights():
                for l in range(2):
                    for name, dram, lst in (
                        ("wq", wqT, wq_sb),
                        ("wk", wkT, wk_sb),
                        ("wv", wvT, wv_sb),
                        ("wo", woT, wo_sb),
                    ):
                        for k in range(NT):
                            t = wpool.tile(
                                [128, H], F16, name=f"{name}{l}{k}", tag=f"{name}{l}{k}"
                            )
                            nc.sync.dma_start(t[:], dram[l, k * 128 : (k + 1) * 128, :])
                            lst[l].append(t)

            bs_t = wpool.tile([128, NT], F32, name="bs", tag="bs")
            bq_t = [wpool.tile([128, NT], F32, name=f"bq{l}", tag=f"bq{l}") for l in range(2)]
            bk_t = [wpool.tile([128, NT], F32, name=f"bk{l}", tag=f"bk{l}") for l in range(2)]
            nc.sync.dma_start(bs_t[:], bs[:, 0].rearrange("(m p) -> p m", p=128))
            for l in range(2):
                nc.sync.dma_start(bq_t[l][:], bq[l, :, 0].rearrange("(m p) -> p m", p=128))
                nc.sync.dma_start(bk_t[l][:], bk[l, :, 0].rearrange("(m p) -> p m", p=128))
            bo2_t = [wpool.tile([1, H], F16, name=f"bo2{l}", tag=f"bo2{l}") for l in range(2)]
            for l in range(2):
                nc.sync.dma_start(bo2_t[l][:], bo2[l])
            mask_t = wpool.tile([128, SB], F16, name="mask", tag="mask")
            nc.sync.dma_start(mask_t[:], mask4[:])
            ident_t = wpool.tile([128, 128], F16, name="ident", tag="ident")
            nc.sync.dma_start(ident_t[:], ident[:])
            ones_row = wpool.tile([1, SB], F16, name="ones", tag="ones")
            nc.vector.memset(ones_row[:], 1.0)

            def attention(l, xsrc, q_tiles, k_tiles, blk, aout):
                """Group-local attention for one 128-gene block; writes
                gene-major normalized output into aout [128, H] fp16."""
                vaug = v_pool.tile([128, NH * (HD + 1)], F16, name="vaug", tag="vaug")
                va = vaug[:].rearrange("p (h c) -> p h c", c=HD + 1)
                for half in range(2):
                    ps = pbig.tile([128, 384], F32, name="pbig", tag="pbig")
                    for k in range(NT):
                        nc.tensor.matmul(
                            ps[:],
                            xsrc[k][:, blk * 128 : (blk + 1) * 128],
                            wv_sb[l][k][:, half * 384 : (half + 1) * 384],
                            start=(k == 0),
                            stop=(k == NT - 1),
                        )
                    nc.vector.tensor_copy(
                        va[:, half * 6 : (half + 1) * 6, 0:HD],
                        ps[:].rearrange("p (h c) -> p h c", c=HD),
                    )
                nc.vector.memset(va[:, :, HD : HD + 1], 1.0)

                # head pairs of equal parity: each scores bank only ever
                # sees one PE row-group (mixing row-groups hangs the HW)
                for h0, h1 in ((0, 2), (1, 3), (4, 6), (5, 7), (8, 10), (9, 11)):
                    r = (h0 % 2) * 64
                    tag = "psl" if r == 0 else "psh"
                    ps_s = psc.tile([128, 256], F32, name=tag, tag=tag)
                    for j, h in enumerate((h0, h1)):
                        t = h // 2
                        nc.tensor.matmul(
                            ps_s[:, j * 128 : (j + 1) * 128],
                            k_tiles[t][r : r + HD, blk * 128 : (blk + 1) * 128],
                            q_tiles[t][r : r + HD, blk * 128 : (blk + 1) * 128],
                            start=True, stop=True, tile_position=(r, 0),
                        )
                    e_t = ea_pool.tile([128, 256], F16, name="e", tag="e")
                    nc.scalar.activation(e_t[:], ps_s[:], AF.Exp, scale=0.125)
                    a_t = ea_pool.tile([128, 256], F16, name="a", tag="a")
                    nc.vector.tensor_mul(a_t[:], e_t[:], mask_t[:, 0:256])
                    ps_o = patt.tile([128, 2 * (HD + 1)], F32, name="patt", tag="patt")
                    po = ps_o[:].rearrange("p (j c) -> p j c", c=HD + 1)
                    for j, h in enumerate((h0, h1)):
                        nc.tensor.matmul(
                            ps_o[:, j * (HD + 1) : (j + 1) * (HD + 1)],
                            a_t[:, j * 128 : (j + 1) * 128],
                            va[:, h, :],
                            start=True, stop=True,
                        )
                    r4 = small_pool.tile([128, 2], F32, name="r4", tag="r4")
                    nc.vector.reciprocal(r4[:], po[:, :, HD])
                    for j, h in enumerate((h0, h1)):
                        nc.scalar.activation(
                            aout[:, h * HD : (h + 1) * HD],
                            po[:, j, 0:HD],
                            AF.Copy,
                            scale=r4[:, j : j + 1],
                        )

            for sb in range(nsb):
                g0 = sb * SB
                xh = []
                for k in range(NT):
                    t = xh_pool.tile([128, SB], F16, name="xh", tag="xh")
                    nc.sync.dma_start(t[:], xt_h[k * 128 : (k + 1) * 128, g0 : g0 + SB])
                    xh.append(t)
                if sb == 0:
                    load_layer_weights()

                # static branch
                for m in range(NT):
                    ps = pbig.tile([128, SB], F32, name="pbig", tag="pbig")
                    for k in range(NT):
                        nc.tensor.matmul(
                            ps[:],
                            ws_sb[k][:, m * 128 : (m + 1) * 128],
                            xh[k][:],
                            start=(k == 0),
                            stop=(k == NT - 1),
                        )
                    st = res_pool.tile([128, SB], F32, name="st", tag="st")
                    nc.scalar.activation(
                        st[:], ps[:], AF.Relu, bias=bs_t[:, m : m + 1], scale=1.0
                    )
                    nc.sync.dma_start(
                        staticT[m * 128 : (m + 1) * 128, g0 : g0 + SB], st[:]
                    )

                xin_h = xh
                xin_f = None
                for l in range(2):
                    # Q, K projections (feature-major)
                    q_tiles, k_tiles = [], []
                    for dest, w_sb, b_t in (
                        (q_tiles, wq_sb[l], bq_t[l]),
                        (k_tiles, wk_sb[l], bk_t[l]),
                    ):
                        for m in range(NT):
                            ps = pbig.tile([128, SB], F32, name="pbig", tag="pbig")
                            for k in range(NT):
                                nc.tensor.matmul(
                                    ps[:],
                                    w_sb[k][:, m * 128 : (m + 1) * 128],
                                    xin_h[k][:],
                                    start=(k == 0),
                                    stop=(k == NT - 1),
                                )
                            qt = qk_pool.tile([128, SB], F16, name="qk", tag="qk")
                            nc.vector.tensor_scalar_add(qt[:], ps[:], b_t[:, m : m + 1])
                            dest.append(qt)

                    af_tiles = [
                        af_pool.tile([128, SB], F16, name="af", tag="af")
                        for _ in range(NT)
                    ]
                    for blk in range(NBLK):
                        aout = aout_pool.tile([128, H], F16, name="aout", tag="aout")
                        attention(l, xin_h, q_tiles, k_tiles, blk, aout)
                        for t in range(NT):
                            ps_t = ptp.tile([128, 128], F16, name="ptp", tag="ptp")
                            nc.tensor.transpose(
                                ps_t[:], aout[:, t * 128 : (t + 1) * 128], ident_t[:]
                            )
                            nc.vector.tensor_copy(
                                af_tiles[t][:, blk * 128 : (blk + 1) * 128], ps_t[:]
                            )

                    if l == 0:
                        # residual base loaded late (off the startup DMA path)
                        xin_f = []
                        for k in range(NT):
                            t = xf_pool.tile([128, SB], F32, name="xf", tag="xf")
                            nc.sync.dma_start(
                                t[:], xt_f[k * 128 : (k + 1) * 128, g0 : g0 + SB]
                            )
                            xin_f.append(t)

                    # output projection (beta folded in on host) + residual
                    new_f, new_h = [], []
                    for m in range(NT):
                        ps = pbig.tile([128, SB], F32, name="pbig", tag="pbig")
                        for k in range(NT):
                            nc.tensor.matmul(
                                ps[:],
                                wo_sb[l][k][:, m * 128 : (m + 1) * 128],
                                af_tiles[k][:],
                                start=(k == 0),
                                stop=False,
                            )
                        nc.tensor.matmul(
                            ps[:],
                            bo2_t[l][:, m * 128 : (m + 1) * 128],
                            ones_row[:],
                            start=False,
                            stop=True,
                        )
                        if l == 0:
                            xnf = x1_pool.tile([128, SB], F32, name="x1f", tag="x1f")
                            nc.vector.tensor_add(xnf[:], ps[:], xin_f[m][:])
                            xnh = x1_pool.tile([128, SB], F16, name="x1h", tag="x1h")
                            nc.scalar.activation(xnh[:], xnf[:], AF.Copy, scale=1.0)
                            new_f.append(xnf)
                            new_h.append(xnh)
                        else:
                            xo = res_pool.tile([128, SB], F32, name="xo", tag="xo")
                            nc.vector.tensor_add(xo[:], ps[:], xin_f[m][:])
                            nc.sync.dma_start(
                                outT[m * 128 : (m + 1) * 128, g0 : g0 + SB], xo[:]
                            )
                    if l == 0:
                        xin_f, xin_h = new_f, new_h

    nc.finalize()
    return nc


def host_prep(inputs: dict, core: int, gpc: int = GPC) -> dict:
    """Slice/transpose/cast inputs for one core."""
    ge = np.asarray(inputs["gene_embeddings"], np.float32)
    Ws = np.asarray(inputs["W_static"], np.float32)
    bs = np.asarray(inputs["b_static"], np.float32)
    Wq = np.asarray(inputs["Wq"], np.float32)
    bq = np.asarray(inputs["bq"], np.float32)
    Wk = np.asarray(inputs["Wk"], np.float32)
    bk = np.asarray(inputs["bk"], np.float32)
    Wv = np.asarray(inputs["Wv"], np.float32)
    bv = np.asarray(inputs["bv"], np.float32)
    Wo = np.asarray(inputs["Wo"], np.float32)
    bo = np.asarray(inputs["bo"], np.float32)
    beta = np.asarray(inputs["beta"], np.float32)

    xs = ge[core * gpc : (core + 1) * gpc].T  # [768, gpc]
    woT = np.stack([(beta[l] * Wo[l].T).astype(np.float16) for l in range(2)])
    bo2 = np.stack(
        [(beta[l] * (Wo[l] @ bv[l] + bo[l])).astype(np.float16)[None, :] for l in range(2)]
    )
    # block-diagonal (8 groups of 16) minus identity, tiled 4x
    m = np.kron(np.eye(8, dtype=np.float16), np.ones((16, 16), np.float16))
    m -= np.eye(128, dtype=np.float16)
    mask4 = np.tile(m, (1, 4))
    return {
        "xt_h": np.ascontiguousarray(xs, dtype=np.float16),
        "xt_f": np.ascontiguousarray(xs, dtype=np.float32),
        "wsT": np.ascontiguousarray(Ws.T, dtype=np.float16),
        "bs": bs.astype(np.float32).reshape(H, 1),
        "wqT": np.ascontiguousarray(Wq.transpose(0, 2, 1), dtype=np.float16),
        "wkT": np.ascontiguousarray(Wk.transpose(0, 2, 1), dtype=np.float16),
        "wvT": np.ascontiguousarray(Wv.transpose(0, 2, 1), dtype=np.float16),
        "woT": np.ascontiguousarray(woT),
        "bq": bq.astype(np.float32).reshape(2, H, 1),
        "bk": bk.astype(np.float32).reshape(2, H, 1),
        "bo2": np.ascontiguousarray(bo2),
        "mask4": np.ascontiguousarray(mask4),
        "ident": np.eye(128, dtype=np.float16),
    }


_CACHED = {}


def _get_program():
    if "nc" not in _CACHED:
        _CACHED["nc"] = build_program(GPC)
    return _CACHED["nc"]


def kernel(**inputs):
    from concourse.bass_utils import run_bass_kernel_spmd

    nc = _get_program()
    in_maps = [host_prep(inputs, c) for c in range(N_CORES)]
    res = run_bass_kernel_spmd(nc, in_maps, list(range(N_CORES)))
    static = np.concatenate([np.asarray(r["staticT"]).T for r in res.results], axis=0)
    x = np.concatenate([np.asarray(r["outT"]).T for r in res.results], axis=0)
    return static.astype(np.float32), x.astype(np.float32)


if __name__ == "__main__":
    nc = build_program(GPC)
    print("build ok")
